# revision 18
# baseline (speedup 1.0000x reference)
"""DiffAttention (dual-branch differential attention) on 8 TRN2 NeuronCores.

Sharding: core c in 0..7 handles batch b = c//4 and kv-head group g = c%4
(4 query heads + 1 kv head per group, REP=4).  Each core computes its heads'
dual-branch attention locally in a transposed layout ([feature, t] on chip so
the head-dim contraction lands on the partition axis), projects through its
slice of wo, then a ReduceScatter over each batch's 4-core group sums the
partial outputs and leaves each core with a 256-row strip of the final
(1024, 2048) output.  The host stitches the strips together.

All matmuls run as float32r (full fp32 storage, fast PE mode).
"""

import sys

sys.path.insert(0, "/opt/trn_rl_repo")

import numpy as np

import concourse.bass as bass
import concourse.mybir as mybir
import concourse.tile as tile
from concourse import bacc

B, T, D = 2, 1024, 2048
H, KV, Dh = 16, 4, 128
REP = H // KV
LAMBDA_INIT = 0.2
ROPE_THETA = 10000.0
SCALE = 1.0 / float(np.sqrt(Dh))

NCORES = 8
GROUPS = [[0, 1, 2, 3], [4, 5, 6, 7]]

F32 = mybir.dt.float32
F32R = mybir.dt.float32r

HPC = 4          # query heads per core
MQ = 2 * HPC     # 8 q-feature chunks of 128 (q1/q2 per head)
NK = D // 128    # 16 contraction chunks for projections
NT = T // 512    # 2 t-chunks of 512
NS = T // 128    # 8 s-chunks of 128
QCOLS = 2 * HPC * Dh   # 1024 per-core q/wo feature columns
KVCOLS = 2 * Dh        # 256 per-core k/v columns


def r(ap):
    return ap.bitcast(F32R)


def _host_tables():
    """Transposed rope tables, causal diag masks, ones/fives columns."""
    inv_freq = 1.0 / (ROPE_THETA ** (np.arange(0, Dh, 2, dtype=np.float64) / Dh))
    freqs = np.arange(T, dtype=np.float64)[:, None] * inv_freq[None, :]  # (T, 64)
    cos = np.cos(freqs).astype(np.float32)
    sin = np.sin(freqs).astype(np.float32)
    rope_c = np.empty((Dh, T), np.float32)
    rope_s = np.empty((Dh, T), np.float32)
    rope_c[:64] = cos.T
    rope_c[64:] = cos.T
    rope_s[:64] = -sin.T
    rope_s[64:] = sin.T

    # mask[r][sp, tp] = 1 where (r*128 + sp) <= tp, for diagonal-crossing blocks
    masks = np.zeros((4, 128, 512), np.float32)
    sp = np.arange(128)[:, None]
    tp = np.arange(512)[None, :]
    for rr in range(4):
        masks[rr] = (rr * 128 + sp <= tp).astype(np.float32)

    ones2 = np.empty((128, 2), np.float32)
    ones2[:, 0] = 1.0
    ones2[:, 1] = 1.0 / LAMBDA_INIT  # 5.0: folds the lambda into 1/denom2
    return rope_c, rope_s, masks, ones2


def _body(tc, ins, outs, with_collective):
    nc = tc.nc
    xT, wq, wk, wv, wo, rope_c_d, rope_s_d, masks_d, ones2_d = ins
    out_d = outs[0]

    cp_cm = tc.tile_pool(name="const", bufs=1)
    cp = cp_cm.__enter__()
    ones2 = cp.tile([128, 2], F32R, name="ones2", tag="ones2")
    mask = [cp.tile([128, 512], F32, name=f"mask{i}", tag=f"mask{i}") for i in range(4)]
    nc.scalar.dma_start(out=ones2[:, :], in_=ones2_d[:, :])
    for i in range(4):
        nc.scalar.dma_start(out=mask[i][:, :], in_=masks_d[i])

    qp_cm = tc.tile_pool(name="qT", bufs=1)
    qp = qp_cm.__enter__()
    qT = [qp.tile([128, T], F32, name=f"qT{m}", tag=f"qT{m}") for m in range(MQ)]
    kT = [qp.tile([128, T], F32, name=f"kT{m}", tag=f"kT{m}") for m in range(2)]
    vt = [qp.tile([128, KVCOLS], F32, name=f"v{i}", tag=f"v{i}") for i in range(NS)]

    # ---------------- phase 1: projections + rope ----------------
    with tc.tile_pool(name="ph1", bufs=1) as ph1, \
         tc.tile_pool(name="ph1w", bufs=2) as ph1w, \
         tc.tile_pool(name="ph1ps", bufs=4, space="PSUM") as ph1ps, \
         tc.tile_pool(name="rotps", bufs=2) as rotp:
        xt = [ph1.tile([128, T], F32R, name=f"xt{k}", tag=f"xt{k}") for k in range(NK)]
        _qs = [nc.gpsimd, nc.sync, nc.scalar]
        for k in range(NK):
            _qs[k % 3].dma_start(out=xt[k][:, :], in_=xT[k * 128:(k + 1) * 128, :])
        wvt = [ph1.tile([128, KVCOLS], F32R, name=f"wv{k}", tag=f"wv{k}") for k in range(NK)]
        for k in range(NK):
            nc.scalar.dma_start(out=wvt[k][:, :], in_=wv[k * 128:(k + 1) * 128, :])
        rope_c = ph1.tile([128, T], F32, name="rope_c", tag="rope_c")
        rope_s = ph1.tile([128, T], F32, name="rope_s", tag="rope_s")
        nc.scalar.dma_start(out=rope_c[:, :], in_=rope_c_d[:, :])
        nc.scalar.dma_start(out=rope_s[:, :], in_=rope_s_d[:, :])

        def rope(dst):
            # dst: [128, T] tile holding one 128-dim rope block in [d, t] layout
            tmp = rotp.tile([128, T], F32, name="rope_tmp", tag="rope_tmp", bufs=2)
            nc.sync.dma_start(out=tmp[0:64, :], in_=dst[64:128, :])
            nc.sync.dma_start(out=tmp[64:128, :], in_=dst[0:64, :])
            nc.vector.tensor_mul(tmp[:, :], tmp[:, :], rope_s[:, :])
            nc.vector.tensor_mul(r(dst[:, :]), dst[:, :], rope_c[:, :])
            nc.vector.tensor_add(r(dst[:, :]), dst[:, :], tmp[:, :])

        # Q^T / K^T: weight m-pairs loaded as [128, 256] tiles, DMAs spread
        # over the sync and scalar HWDGE queues
        def proj_pair(w_dram, dsts, mp):
            wt = [ph1w.tile([128, 256], F32R, name=f"wp{k}", tag=f"wp{k}") for k in range(NK)]
            for k in range(NK):
                eng = nc.sync if k % 2 == 0 else nc.scalar
                eng.dma_start(
                    out=wt[k][:, :],
                    in_=w_dram[k * 128:(k + 1) * 128, mp * 256:(mp + 1) * 256],
                )
            for mi in range(2):
                dst = dsts[mi]
                msl = slice(mi * 128, (mi + 1) * 128)
                for t in range(NT):
                    ps = ph1ps.tile([128, 512], F32, name="proj_ps", tag="proj_ps", bufs=4)
                    for k in range(NK):
                        nc.tensor.matmul(
                            ps[:, :], wt[k][:, msl], xt[k][:, t * 512:(t + 1) * 512],
                            start=(k == 0), stop=(k == NK - 1),
                        )
                    nc.scalar.copy(r(dst[:, t * 512:(t + 1) * 512]), ps[:, :])
                rope(dst)

        for mp in range(MQ // 2):
            proj_pair(wq, qT[2 * mp:2 * mp + 2], mp)
        proj_pair(wk, kT, 0)

        # V in natural [t, dv] layout: lhsT = x^T chunk, rhs = wv chunk
        for i in range(NS):
            ps = ph1ps.tile([128, KVCOLS], F32, name="v_ps", tag="v_ps", bufs=2)
            for k in range(NK):
                nc.tensor.matmul(
                    ps[:, :],
                    xt[k][:, i * 128:(i + 1) * 128],
                    wvt[k][:, :],
                    start=(k == 0), stop=(k == NK - 1),
                )
            nc.scalar.copy(r(vt[i][:, :]), ps[:, :])

    # ---------------- phase 2: dual-branch attention ----------------
    op_cm = tc.tile_pool(name="oT", bufs=1)
    op = op_cm.__enter__()
    oT = [op.tile([128, T], F32, name=f"oT{i}", tag=f"oT{i}") for i in range(MQ)]

    with tc.tile_pool(name="pP", bufs=1) as pP, \
         tc.tile_pool(name="pR", bufs=2) as pR, \
         tc.tile_pool(name="scps", bufs=3, space="PSUM") as scps, \
         tc.tile_pool(name="dps", bufs=2, space="PSUM") as dpsp, \
         tc.tile_pool(name="avps", bufs=2, space="PSUM") as avpsp:

        def emit_branch(h, t, j, pt):
            # scores + exp + mask + denominator for one branch
            s_lim = 4 * (t + 1)
            tsl = slice(t * 512, (t + 1) * 512)
            qt = qT[2 * h + j]
            for si in range(s_lim):
                ps = scps.tile([128, 512], F32, name="sc_ps", tag="sc_ps", bufs=3)
                nc.tensor.matmul(
                    ps[:, :],
                    r(kT[j][:, si * 128:(si + 1) * 128]),
                    r(qt[:, tsl]),
                    start=True, stop=True,
                )
                nc.scalar.activation(
                    r(pt[j][si][:, :]), ps[:, :],
                    mybir.ActivationFunctionType.Exp, scale=SCALE,
                )
                rr = si - 4 * t
                if 0 <= rr < 4:
                    nc.vector.tensor_mul(r(pt[j][si][:, :]), pt[j][si][:, :], mask[rr][:, :])
            dps = dpsp.tile([1, 512], F32, name="d_ps", tag="d_ps", bufs=2)
            for si in range(s_lim):
                nc.tensor.matmul(
                    dps[:, :], ones2[:, 0:1], r(pt[j][si][:, :]),
                    start=(si == 0), stop=(si == s_lim - 1),
                )
            return dps

        def emit_ratio(h, t, d1ps, d2ps):
            # rb2 = lambda * d1/d2 broadcast, rb1 = 1/d1 broadcast
            rec1 = pR.tile([1, 512], F32, name="rec1", tag="rec1", bufs=2)
            nc.vector.reciprocal_approx_fast(rec1[:, :], d1ps[:, :])
            rec2 = pR.tile([1, 512], F32, name="rec2", tag="rec2", bufs=2)
            nc.vector.reciprocal_approx_fast(rec2[:, :], d2ps[:, :])
            c2 = pR.tile([1, 512], F32, name="c2", tag="c2", bufs=2)
            nc.vector.tensor_mul(c2[:, :], rec2[:, :], d1ps[:, :])
            nc.vector.tensor_scalar_mul(c2[:, :], c2[:, :], LAMBDA_INIT)
            rb1 = pR.tile([128, 512], F32, name="rb1", tag="rb1", bufs=2)
            nc.gpsimd.partition_broadcast(rb1[:, :], rec1[:, :])
            rb2 = pR.tile([128, 512], F32, name="rb2", tag="rb2", bufs=2)
            nc.gpsimd.partition_broadcast(rb2[:, :], c2[:, :])
            return rb1, rb2

        def emit_norm_av(h, t, pt, rb):
            # Pd = P1 - (lambda d1/d2) P2  (2 DVE ops per block), AV matmul,
            # then oT scaled by 1/d1 at copy-out
            rb1, rb2 = rb
            s_lim = 4 * (t + 1)
            tsl = slice(t * 512, (t + 1) * 512)
            for si in range(s_lim):
                nc.vector.tensor_mul(r(pt[1][si][:, :]), pt[1][si][:, :], rb2[:, :])
                nc.vector.tensor_sub(r(pt[0][si][:, :]), pt[0][si][:, :], pt[1][si][:, :])
            for dv in range(2):
                avps = avpsp.tile([128, 512], F32, name="av_ps", tag="av_ps", bufs=2)
                for si in range(s_lim):
                    nc.tensor.matmul(
                        avps[:, :],
                        r(vt[si][:, dv * 128:(dv + 1) * 128]),
                        r(pt[0][si][:, :]),
                        start=(si == 0), stop=(si == s_lim - 1),
                    )
                nc.vector.tensor_mul(r(oT[2 * h + dv][:, tsl]), avps[:, :], rb1[:, :])

        # software pipeline: AV of round r-1 slots after branch 0 of round r
        rounds = [(h, t) for h in range(HPC) for t in range(NT)]
        prev = None
        for (h, t) in rounds:
            s_lim = 4 * (t + 1)
            pt = [[pP.tile([128, 512], F32, name=f"p{j}_{si}", tag=f"p{j}_{si}", bufs=2)
                   for si in range(s_lim)] for j in range(2)]
            d1 = emit_branch(h, t, 0, pt)
            if prev is not None:
                emit_norm_av(*prev)
            d2 = emit_branch(h, t, 1, pt)
            rb = emit_ratio(h, t, d1, d2)
            prev = (h, t, pt, rb)
        emit_norm_av(*prev)

    # ---------------- phase 3: wo projection ----------------
    with tc.tile_pool(name="drampool", bufs=1, space="DRAM") as dram, \
         tc.tile_pool(name="wopool", bufs=2) as wop, \
         tc.tile_pool(name="wobounce", bufs=4) as wob, \
         tc.tile_pool(name="wops", bufs=4, space="PSUM") as wopsp:
        partial = [dram.tile([T, 512], F32, name=f"partial{n}", tag=f"partial{n}")
                   for n in range(4)]
        rs_out = [dram.tile([T // 4, 512], F32, name=f"rs_out{n}", tag=f"rs_out{n}")
                  for n in range(4)]
        for n in range(4):
            nsl = slice(n * 512, (n + 1) * 512)
            won = [wop.tile([128, 512], F32R, name=f"wo{fi}", tag=f"wo{fi}", bufs=2)
                   for fi in range(MQ)]
            for fi in range(MQ):
                nc.sync.dma_start(
                    out=won[fi][:, :],
                    in_=wo[fi * 128:(fi + 1) * 128, nsl],
                )
            for tm in range(NS):
                ps = wopsp.tile([128, 512], F32, name="wo_ps", tag="wo_ps", bufs=4)
                for fi in range(MQ):
                    nc.tensor.matmul(
                        ps[:, :],
                        r(oT[fi][:, tm * 128:(tm + 1) * 128]),
                        won[fi][:, :],
                        start=(fi == 0), stop=(fi == MQ - 1),
                    )
                bt = wob.tile([128, 512], F32, name="wo_b", tag="wo_b", bufs=4)
                nc.scalar.copy(bt[:, :], ps[:, :])
                if with_collective:
                    nc.sync.dma_start(
                        out=partial[n][tm * 128:(tm + 1) * 128, :],
                        in_=bt[:, :],
                    )
                else:
                    nc.sync.dma_start(
                        out=out_d[tm * 128:(tm + 1) * 128, nsl],
                        in_=bt[:, :],
                    )
            if with_collective:
                # overlap: reduce-scatter this dout column block while the
                # next block's matmuls run; alternate queues so the two CC
                # rings work concurrently
                nc.gpsimd.collective_compute(
                    "ReduceScatter",
                    mybir.AluOpType.add,
                    replica_groups=GROUPS,
                    ins=[partial[n][:, :].opt()],
                    outs=[rs_out[n][:, :].opt()],
                )
                nc.gpsimd.dma_start(out=out_d[:, nsl], in_=rs_out[n][:, :])

    op_cm.__exit__(None, None, None)
    qp_cm.__exit__(None, None, None)
    cp_cm.__exit__(None, None, None)


def build(with_collective=True):
    nc = bacc.Bacc(
        "TRN2",
        target_bir_lowering=False,
        debug=False,
        num_devices=NCORES if with_collective else 1,
    )
    ins = [
        nc.dram_tensor("xT", [D, T], F32R, kind="ExternalInput").ap(),
        nc.dram_tensor("wq", [D, QCOLS], F32R, kind="ExternalInput").ap(),
        nc.dram_tensor("wk", [D, KVCOLS], F32R, kind="ExternalInput").ap(),
        nc.dram_tensor("wv", [D, KVCOLS], F32R, kind="ExternalInput").ap(),
        nc.dram_tensor("wo", [QCOLS, D], F32R, kind="ExternalInput").ap(),
        nc.dram_tensor("rope_c", [128, T], F32, kind="ExternalInput").ap(),
        nc.dram_tensor("rope_s", [128, T], F32, kind="ExternalInput").ap(),
        nc.dram_tensor("masks", [4, 128, 512], F32, kind="ExternalInput").ap(),
        nc.dram_tensor("ones2", [128, 2], F32R, kind="ExternalInput").ap(),
    ]
    oshape = [T // 4, D] if with_collective else [T, D]
    outs = [nc.dram_tensor("out", oshape, F32, kind="ExternalOutput").ap()]
    with tile.TileContext(nc) as tc:
        _body(tc, ins, outs, with_collective)
    nc.compile()
    return nc


def in_maps(x, wq, wk, wv, wo):
    rope_c, rope_s, masks, ones2 = _host_tables()
    x = np.asarray(x, np.float32)
    wq = np.asarray(wq, np.float32)
    wk = np.asarray(wk, np.float32)
    wv = np.asarray(wv, np.float32)
    wo = np.asarray(wo, np.float32)
    maps = []
    for c in range(NCORES):
        b, g = c // 4, c % 4
        maps.append({
            "xT": np.ascontiguousarray(x[b].T),
            "wq": np.ascontiguousarray(wq[:, g * QCOLS:(g + 1) * QCOLS]),
            "wk": np.ascontiguousarray(wk[:, g * KVCOLS:(g + 1) * KVCOLS]),
            "wv": np.ascontiguousarray(wv[:, g * KVCOLS:(g + 1) * KVCOLS]),
            "wo": np.ascontiguousarray(wo[g * QCOLS:(g + 1) * QCOLS, :]),
            "rope_c": rope_c,
            "rope_s": rope_s,
            "masks": masks,
            "ones2": ones2,
        })
    return maps


_NC_CACHE = {}


def get_nc(with_collective=True):
    key = bool(with_collective)
    if key not in _NC_CACHE:
        _NC_CACHE[key] = build(with_collective)
    return _NC_CACHE[key]


def kernel(x, wq, wk, wv, wo):
    from concourse.bass_utils import run_bass_kernel_spmd

    nc = get_nc(True)
    maps = in_maps(x, wq, wk, wv, wo)
    res = run_bass_kernel_spmd(nc, maps, core_ids=list(range(NCORES)))
    out = np.empty((B, T, D), np.float32)
    for c in range(NCORES):
        b, g = c // 4, c % 4
        out[b, g * (T // 4):(g + 1) * (T // 4), :] = res.results[c]["out"]
    return out


# revision 20
# speedup vs baseline: 1.0787x; 1.0787x over previous
"""DiffAttention (dual-branch differential attention) on 8 TRN2 NeuronCores.

Sharding: core c in 0..7 handles batch b = c//4 and kv-head group g = c%4
(4 query heads + 1 kv head per group, REP=4).  Each core computes its heads'
dual-branch attention locally in a transposed layout ([feature, t] on chip so
the head-dim contraction lands on the partition axis), projects through its
slice of wo, then a ReduceScatter over each batch's 4-core group sums the
partial outputs and leaves each core with a 256-row strip of the final
(1024, 2048) output.  The host stitches the strips together.

All matmuls run as float32r (full fp32 storage, fast PE mode).
"""

import sys

sys.path.insert(0, "/opt/trn_rl_repo")

import numpy as np

import concourse.bass as bass
import concourse.mybir as mybir
import concourse.tile as tile
from concourse import bacc

B, T, D = 2, 1024, 2048
H, KV, Dh = 16, 4, 128
REP = H // KV
LAMBDA_INIT = 0.2
ROPE_THETA = 10000.0
SCALE = 1.0 / float(np.sqrt(Dh))

NCORES = 8
GROUPS = [[0, 1, 2, 3], [4, 5, 6, 7]]

F32 = mybir.dt.float32
F32R = mybir.dt.float32r

HPC = 4          # query heads per core
MQ = 2 * HPC     # 8 q-feature chunks of 128 (q1/q2 per head)
NK = D // 128    # 16 contraction chunks for projections
NT = T // 512    # 2 t-chunks of 512
NS = T // 128    # 8 s-chunks of 128
QCOLS = 2 * HPC * Dh   # 1024 per-core q/wo feature columns
KVCOLS = 2 * Dh        # 256 per-core k/v columns


def r(ap):
    return ap.bitcast(F32R)


def _host_tables():
    """Transposed rope tables, causal diag masks, ones/fives columns."""
    inv_freq = 1.0 / (ROPE_THETA ** (np.arange(0, Dh, 2, dtype=np.float64) / Dh))
    freqs = np.arange(T, dtype=np.float64)[:, None] * inv_freq[None, :]  # (T, 64)
    cos = np.cos(freqs).astype(np.float32)
    sin = np.sin(freqs).astype(np.float32)
    rope_c = np.empty((Dh, T), np.float32)
    rope_s = np.empty((Dh, T), np.float32)
    rope_c[:64] = cos.T
    rope_c[64:] = cos.T
    rope_s[:64] = -sin.T
    rope_s[64:] = sin.T

    # mask[r][sp, tp] = 1 where (r*128 + sp) <= tp, for diagonal-crossing blocks
    masks = np.zeros((4, 128, 512), np.float32)
    sp = np.arange(128)[:, None]
    tp = np.arange(512)[None, :]
    for rr in range(4):
        masks[rr] = (rr * 128 + sp <= tp).astype(np.float32)

    ones2 = np.empty((128, 2), np.float32)
    ones2[:, 0] = 1.0
    ones2[:, 1] = 1.0 / LAMBDA_INIT  # 5.0: folds the lambda into 1/denom2
    return rope_c, rope_s, masks, ones2


def _body(tc, ins, outs, with_collective):
    nc = tc.nc
    xT, wq, wk, wv, wo, rope_c_d, rope_s_d, masks_d, ones2_d = ins
    out_d = outs[0]

    cp_cm = tc.tile_pool(name="const", bufs=1)
    cp = cp_cm.__enter__()
    ones2 = cp.tile([128, 2], F32R, name="ones2", tag="ones2")
    mask = [cp.tile([128, 512], F32, name=f"mask{i}", tag=f"mask{i}") for i in range(4)]

    qp_cm = tc.tile_pool(name="qT", bufs=1)
    qp = qp_cm.__enter__()
    qT = [qp.tile([128, T], F32, name=f"qT{m}", tag=f"qT{m}") for m in range(MQ)]
    kT = [qp.tile([128, T], F32, name=f"kT{m}", tag=f"kT{m}") for m in range(2)]
    vt = [qp.tile([128, KVCOLS], F32, name=f"v{i}", tag=f"v{i}") for i in range(NS)]

    # ---------------- phase 1: projections + rope ----------------
    with tc.tile_pool(name="ph1", bufs=1) as ph1, \
         tc.tile_pool(name="ph1w", bufs=2) as ph1w, \
         tc.tile_pool(name="ph1ps", bufs=4, space="PSUM") as ph1ps, \
         tc.tile_pool(name="rotps", bufs=2) as rotp:
        xt = [ph1.tile([128, T], F32R, name=f"xt{k}", tag=f"xt{k}") for k in range(NK)]
        _qs = [nc.gpsimd, nc.sync, nc.scalar]

        def load_xt():
            for k in range(NK):
                _qs[k % 3].dma_start(out=xt[k][:, :], in_=xT[k * 128:(k + 1) * 128, :])
        wvt = [ph1.tile([128, KVCOLS], F32R, name=f"wv{k}", tag=f"wv{k}") for k in range(NK)]
        for k in range(NK):
            nc.gpsimd.dma_start(out=wvt[k][:, :], in_=wv[k * 128:(k + 1) * 128, :])
        rope_c = ph1.tile([128, T], F32, name="rope_c", tag="rope_c")
        rope_s = ph1.tile([128, T], F32, name="rope_s", tag="rope_s")
        nc.gpsimd.dma_start(out=rope_c[:, :], in_=rope_c_d[:, :])
        nc.gpsimd.dma_start(out=rope_s[:, :], in_=rope_s_d[:, :])
        nc.gpsimd.dma_start(out=ones2[:, :], in_=ones2_d[:, :])
        for i in range(4):
            nc.gpsimd.dma_start(out=mask[i][:, :], in_=masks_d[i])

        def rope(dst):
            # dst: [128, T] tile holding one 128-dim rope block in [d, t] layout
            tmp = rotp.tile([128, T], F32, name="rope_tmp", tag="rope_tmp", bufs=2)
            nc.sync.dma_start(out=tmp[0:64, :], in_=dst[64:128, :])
            nc.sync.dma_start(out=tmp[64:128, :], in_=dst[0:64, :])
            nc.vector.tensor_mul(tmp[:, :], tmp[:, :], rope_s[:, :])
            nc.vector.tensor_mul(r(dst[:, :]), dst[:, :], rope_c[:, :])
            nc.vector.tensor_add(r(dst[:, :]), dst[:, :], tmp[:, :])

        # Q^T / K^T: weight m-pairs loaded as [128, 256] tiles, DMAs spread
        # over the sync and scalar HWDGE queues
        def load_pair(w_dram, mp):
            wt = [ph1w.tile([128, 256], F32R, name=f"wp{k}", tag=f"wp{k}") for k in range(NK)]
            for k in range(NK):
                eng = nc.sync if k % 2 == 0 else nc.scalar
                eng.dma_start(
                    out=wt[k][:, :],
                    in_=w_dram[k * 128:(k + 1) * 128, mp * 256:(mp + 1) * 256],
                )
            return wt

        def proj_pair(w_dram, dsts, mp, wt):
            for mi in range(2):
                dst = dsts[mi]
                msl = slice(mi * 128, (mi + 1) * 128)
                for t in range(NT):
                    ps = ph1ps.tile([128, 512], F32, name="proj_ps", tag="proj_ps", bufs=4)
                    for k in range(NK):
                        nc.tensor.matmul(
                            ps[:, :], wt[k][:, msl], xt[k][:, t * 512:(t + 1) * 512],
                            start=(k == 0), stop=(k == NK - 1),
                        )
                    nc.scalar.copy(r(dst[:, t * 512:(t + 1) * 512]), ps[:, :])
                rope(dst)

        pair_specs = [(wq, qT[2 * mp:2 * mp + 2], mp) for mp in range(MQ // 2)]
        pair_specs.append((wk, kT, 0))
        wt_next = load_pair(pair_specs[0][0], pair_specs[0][2])
        load_xt()
        for pi, (w_dram, dsts, mp) in enumerate(pair_specs):
            wt = wt_next
            if pi + 1 < len(pair_specs):
                wt_next = load_pair(pair_specs[pi + 1][0], pair_specs[pi + 1][2])
            proj_pair(w_dram, dsts, mp, wt)

        # V in natural [t, dv] layout: lhsT = x^T chunk, rhs = wv chunk
        for i in range(NS):
            ps = ph1ps.tile([128, KVCOLS], F32, name="v_ps", tag="v_ps", bufs=2)
            for k in range(NK):
                nc.tensor.matmul(
                    ps[:, :],
                    xt[k][:, i * 128:(i + 1) * 128],
                    wvt[k][:, :],
                    start=(k == 0), stop=(k == NK - 1),
                )
            nc.scalar.copy(r(vt[i][:, :]), ps[:, :])

    # ---------------- phase 2: dual-branch attention ----------------
    op_cm = tc.tile_pool(name="oT", bufs=1)
    op = op_cm.__enter__()
    oT = [op.tile([128, T], F32, name=f"oT{i}", tag=f"oT{i}") for i in range(MQ)]

    with tc.tile_pool(name="pP", bufs=1) as pP, \
         tc.tile_pool(name="pR", bufs=2) as pR, \
         tc.tile_pool(name="scps", bufs=3, space="PSUM") as scps, \
         tc.tile_pool(name="dps", bufs=2, space="PSUM") as dpsp, \
         tc.tile_pool(name="avps", bufs=2, space="PSUM") as avpsp:

        def emit_branch(h, t, j, pt):
            # scores + exp + mask + denominator for one branch
            s_lim = 4 * (t + 1)
            tsl = slice(t * 512, (t + 1) * 512)
            qt = qT[2 * h + j]
            for si in range(s_lim):
                ps = scps.tile([128, 512], F32, name="sc_ps", tag="sc_ps", bufs=3)
                nc.tensor.matmul(
                    ps[:, :],
                    r(kT[j][:, si * 128:(si + 1) * 128]),
                    r(qt[:, tsl]),
                    start=True, stop=True,
                )
                nc.scalar.activation(
                    r(pt[j][si][:, :]), ps[:, :],
                    mybir.ActivationFunctionType.Exp, scale=SCALE,
                )
                rr = si - 4 * t
                if 0 <= rr < 4:
                    nc.vector.tensor_mul(r(pt[j][si][:, :]), pt[j][si][:, :], mask[rr][:, :])
            dps = dpsp.tile([1, 512], F32, name="d_ps", tag="d_ps", bufs=2)
            for si in range(s_lim):
                nc.tensor.matmul(
                    dps[:, :], ones2[:, 0:1], r(pt[j][si][:, :]),
                    start=(si == 0), stop=(si == s_lim - 1),
                )
            return dps

        def emit_ratio(h, t, d1ps, d2ps):
            # rb2 = lambda * d1/d2 broadcast, rb1 = 1/d1 broadcast
            rec1 = pR.tile([1, 512], F32, name="rec1", tag="rec1", bufs=2)
            nc.vector.reciprocal_approx_fast(rec1[:, :], d1ps[:, :])
            rec2 = pR.tile([1, 512], F32, name="rec2", tag="rec2", bufs=2)
            nc.vector.reciprocal_approx_fast(rec2[:, :], d2ps[:, :])
            c2 = pR.tile([1, 512], F32, name="c2", tag="c2", bufs=2)
            nc.vector.tensor_mul(c2[:, :], rec2[:, :], d1ps[:, :])
            nc.vector.tensor_scalar_mul(c2[:, :], c2[:, :], LAMBDA_INIT)
            rb1 = pR.tile([128, 512], F32, name="rb1", tag="rb1", bufs=2)
            nc.gpsimd.partition_broadcast(rb1[:, :], rec1[:, :])
            rb2 = pR.tile([128, 512], F32, name="rb2", tag="rb2", bufs=2)
            nc.gpsimd.partition_broadcast(rb2[:, :], c2[:, :])
            return rb1, rb2

        def emit_norm_av(h, t, pt, rb):
            # Pd = P1 - (lambda d1/d2) P2  (2 DVE ops per block), AV matmul,
            # then oT scaled by 1/d1 at copy-out
            rb1, rb2 = rb
            s_lim = 4 * (t + 1)
            tsl = slice(t * 512, (t + 1) * 512)
            for si in range(s_lim):
                nc.vector.tensor_mul(r(pt[1][si][:, :]), pt[1][si][:, :], rb2[:, :])
                nc.vector.tensor_sub(r(pt[0][si][:, :]), pt[0][si][:, :], pt[1][si][:, :])
            for dv in range(2):
                avps = avpsp.tile([128, 512], F32, name="av_ps", tag="av_ps", bufs=2)
                for si in range(s_lim):
                    nc.tensor.matmul(
                        avps[:, :],
                        r(vt[si][:, dv * 128:(dv + 1) * 128]),
                        r(pt[0][si][:, :]),
                        start=(si == 0), stop=(si == s_lim - 1),
                    )
                nc.vector.tensor_mul(r(oT[2 * h + dv][:, tsl]), avps[:, :], rb1[:, :])

        # software pipeline: AV of round r-1 slots after branch 0 of round r
        rounds = [(h, t) for h in range(HPC) for t in range(NT)]
        prev = None
        for (h, t) in rounds:
            s_lim = 4 * (t + 1)
            pt = [[pP.tile([128, 512], F32, name=f"p{j}_{si}", tag=f"p{j}_{si}", bufs=2)
                   for si in range(s_lim)] for j in range(2)]
            d1 = emit_branch(h, t, 0, pt)
            if prev is not None:
                emit_norm_av(*prev)
            d2 = emit_branch(h, t, 1, pt)
            rb = emit_ratio(h, t, d1, d2)
            prev = (h, t, pt, rb)
        emit_norm_av(*prev)

    # ---------------- phase 3: wo projection ----------------
    with tc.tile_pool(name="drampool", bufs=1, space="DRAM") as dram, \
         tc.tile_pool(name="wopool", bufs=2) as wop, \
         tc.tile_pool(name="wobounce", bufs=4) as wob, \
         tc.tile_pool(name="wops", bufs=4, space="PSUM") as wopsp:
        partial = [dram.tile([T, 512], F32, name=f"partial{n}", tag=f"partial{n}")
                   for n in range(4)]
        rs_out = [dram.tile([T // 4, 512], F32, name=f"rs_out{n}", tag=f"rs_out{n}")
                  for n in range(4)]
        for n in range(4):
            nsl = slice(n * 512, (n + 1) * 512)
            won = [wop.tile([128, 512], F32R, name=f"wo{fi}", tag=f"wo{fi}", bufs=2)
                   for fi in range(MQ)]
            for fi in range(MQ):
                nc.sync.dma_start(
                    out=won[fi][:, :],
                    in_=wo[fi * 128:(fi + 1) * 128, nsl],
                )
            for tm in range(NS):
                ps = wopsp.tile([128, 512], F32, name="wo_ps", tag="wo_ps", bufs=4)
                for fi in range(MQ):
                    nc.tensor.matmul(
                        ps[:, :],
                        r(oT[fi][:, tm * 128:(tm + 1) * 128]),
                        won[fi][:, :],
                        start=(fi == 0), stop=(fi == MQ - 1),
                    )
                bt = wob.tile([128, 512], F32, name="wo_b", tag="wo_b", bufs=4)
                nc.scalar.copy(bt[:, :], ps[:, :])
                if with_collective:
                    nc.sync.dma_start(
                        out=partial[n][tm * 128:(tm + 1) * 128, :],
                        in_=bt[:, :],
                    )
                else:
                    nc.sync.dma_start(
                        out=out_d[tm * 128:(tm + 1) * 128, nsl],
                        in_=bt[:, :],
                    )
            if with_collective:
                # overlap: reduce-scatter this dout column block while the
                # next block's matmuls run; alternate queues so the two CC
                # rings work concurrently
                nc.gpsimd.collective_compute(
                    "ReduceScatter",
                    mybir.AluOpType.add,
                    replica_groups=GROUPS,
                    ins=[partial[n][:, :].opt()],
                    outs=[rs_out[n][:, :].opt()],
                )
                nc.gpsimd.dma_start(out=out_d[:, nsl], in_=rs_out[n][:, :])

    op_cm.__exit__(None, None, None)
    qp_cm.__exit__(None, None, None)
    cp_cm.__exit__(None, None, None)


def build(with_collective=True):
    nc = bacc.Bacc(
        "TRN2",
        target_bir_lowering=False,
        debug=False,
        num_devices=NCORES if with_collective else 1,
    )
    ins = [
        nc.dram_tensor("xT", [D, T], F32R, kind="ExternalInput").ap(),
        nc.dram_tensor("wq", [D, QCOLS], F32R, kind="ExternalInput").ap(),
        nc.dram_tensor("wk", [D, KVCOLS], F32R, kind="ExternalInput").ap(),
        nc.dram_tensor("wv", [D, KVCOLS], F32R, kind="ExternalInput").ap(),
        nc.dram_tensor("wo", [QCOLS, D], F32R, kind="ExternalInput").ap(),
        nc.dram_tensor("rope_c", [128, T], F32, kind="ExternalInput").ap(),
        nc.dram_tensor("rope_s", [128, T], F32, kind="ExternalInput").ap(),
        nc.dram_tensor("masks", [4, 128, 512], F32, kind="ExternalInput").ap(),
        nc.dram_tensor("ones2", [128, 2], F32R, kind="ExternalInput").ap(),
    ]
    oshape = [T // 4, D] if with_collective else [T, D]
    outs = [nc.dram_tensor("out", oshape, F32, kind="ExternalOutput").ap()]
    with tile.TileContext(nc) as tc:
        _body(tc, ins, outs, with_collective)
    nc.compile()
    return nc


def in_maps(x, wq, wk, wv, wo):
    rope_c, rope_s, masks, ones2 = _host_tables()
    x = np.asarray(x, np.float32)
    wq = np.asarray(wq, np.float32)
    wk = np.asarray(wk, np.float32)
    wv = np.asarray(wv, np.float32)
    wo = np.asarray(wo, np.float32)
    maps = []
    for c in range(NCORES):
        b, g = c // 4, c % 4
        maps.append({
            "xT": np.ascontiguousarray(x[b].T),
            "wq": np.ascontiguousarray(wq[:, g * QCOLS:(g + 1) * QCOLS]),
            "wk": np.ascontiguousarray(wk[:, g * KVCOLS:(g + 1) * KVCOLS]),
            "wv": np.ascontiguousarray(wv[:, g * KVCOLS:(g + 1) * KVCOLS]),
            "wo": np.ascontiguousarray(wo[g * QCOLS:(g + 1) * QCOLS, :]),
            "rope_c": rope_c,
            "rope_s": rope_s,
            "masks": masks,
            "ones2": ones2,
        })
    return maps


_NC_CACHE = {}


def get_nc(with_collective=True):
    key = bool(with_collective)
    if key not in _NC_CACHE:
        _NC_CACHE[key] = build(with_collective)
    return _NC_CACHE[key]


def kernel(x, wq, wk, wv, wo):
    from concourse.bass_utils import run_bass_kernel_spmd

    nc = get_nc(True)
    maps = in_maps(x, wq, wk, wv, wo)
    res = run_bass_kernel_spmd(nc, maps, core_ids=list(range(NCORES)))
    out = np.empty((B, T, D), np.float32)
    for c in range(NCORES):
        b, g = c // 4, c % 4
        out[b, g * (T // 4):(g + 1) * (T // 4), :] = res.results[c]["out"]
    return out


# revision 21
# speedup vs baseline: 1.2582x; 1.1665x over previous
"""DiffAttention (dual-branch differential attention) on 8 TRN2 NeuronCores.

Sharding: core c in 0..7 handles batch b = c//4 and kv-head group g = c%4
(4 query heads + 1 kv head per group, REP=4).  Each core computes its heads'
dual-branch attention locally in a transposed layout ([feature, t] on chip so
the head-dim contraction lands on the partition axis), projects through its
slice of wo, then a ReduceScatter over each batch's 4-core group sums the
partial outputs and leaves each core with a 256-row strip of the final
(1024, 2048) output.  The host stitches the strips together.

All matmuls run as float32r (full fp32 storage, fast PE mode).
"""

import sys

sys.path.insert(0, "/opt/trn_rl_repo")

import numpy as np

import concourse.bass as bass
import concourse.mybir as mybir
import concourse.tile as tile
from concourse import bacc

B, T, D = 2, 1024, 2048
H, KV, Dh = 16, 4, 128
REP = H // KV
LAMBDA_INIT = 0.2
ROPE_THETA = 10000.0
SCALE = 1.0 / float(np.sqrt(Dh))

NCORES = 8
GROUPS = [[0, 1, 2, 3], [4, 5, 6, 7]]

F32 = mybir.dt.float32
F32R = mybir.dt.float32r
BF16 = mybir.dt.bfloat16

HPC = 4          # query heads per core
MQ = 2 * HPC     # 8 q-feature chunks of 128 (q1/q2 per head)
NK = D // 128    # 16 contraction chunks for projections
NT = T // 512    # 2 t-chunks of 512
NS = T // 128    # 8 s-chunks of 128
QCOLS = 2 * HPC * Dh   # 1024 per-core q/wo feature columns
KVCOLS = 2 * Dh        # 256 per-core k/v columns


def r(ap):
    return ap.bitcast(F32R)


def _host_tables():
    """Transposed rope tables, causal diag masks, ones/fives columns."""
    inv_freq = 1.0 / (ROPE_THETA ** (np.arange(0, Dh, 2, dtype=np.float64) / Dh))
    freqs = np.arange(T, dtype=np.float64)[:, None] * inv_freq[None, :]  # (T, 64)
    cos = np.cos(freqs).astype(np.float32)
    sin = np.sin(freqs).astype(np.float32)
    rope_c = np.empty((Dh, T), np.float32)
    rope_s = np.empty((Dh, T), np.float32)
    rope_c[:64] = cos.T
    rope_c[64:] = cos.T
    rope_s[:64] = -sin.T
    rope_s[64:] = sin.T

    # mask[r][sp, tp] = 1 where (r*128 + sp) <= tp, for diagonal-crossing blocks
    masks = np.zeros((4, 128, 512), np.float32)
    sp = np.arange(128)[:, None]
    tp = np.arange(512)[None, :]
    for rr in range(4):
        masks[rr] = (rr * 128 + sp <= tp).astype(np.float32)

    ones2 = np.empty((128, 2), np.float32)
    ones2[:, 0] = 1.0
    ones2[:, 1] = 1.0 / LAMBDA_INIT  # 5.0: folds the lambda into 1/denom2
    return rope_c, rope_s, masks, ones2


def _body(tc, ins, outs, with_collective):
    nc = tc.nc
    xT, wq, wk, wv, wo, rope_c_d, rope_s_d, masks_d, ones2_d = ins
    out_d = outs[0]

    cp_cm = tc.tile_pool(name="const", bufs=1)
    cp = cp_cm.__enter__()
    ones2 = cp.tile([128, 2], F32R, name="ones2", tag="ones2")

    qp_cm = tc.tile_pool(name="qT", bufs=1)
    qp = qp_cm.__enter__()
    qT = [qp.tile([128, T], F32, name=f"qT{m}", tag=f"qT{m}") for m in range(MQ)]
    kT = [qp.tile([128, T], F32, name=f"kT{m}", tag=f"kT{m}") for m in range(2)]
    vt = [qp.tile([128, KVCOLS], F32, name=f"v{i}", tag=f"v{i}") for i in range(NS)]

    # ---------------- phase 1: projections + rope ----------------
    with tc.tile_pool(name="ph1", bufs=1) as ph1, \
         tc.tile_pool(name="ph1w", bufs=2) as ph1w, \
         tc.tile_pool(name="ph1ps", bufs=4, space="PSUM") as ph1ps, \
         tc.tile_pool(name="rotps", bufs=2) as rotp:
        # x^T staged as one [128, 16*1024] tile, 4 strided DMAs of 4 k-chunks
        # each (descriptor-count, not bandwidth, limits the DMA queues)
        xt_all = ph1.tile([128, NK * T], F32R, name="xt_all", tag="xt_all")
        xt = [xt_all[:, k * T:(k + 1) * T] for k in range(NK)]
        _qs = [nc.sync, nc.scalar, nc.gpsimd, nc.sync]

        def load_xt():
            for c in range(4):
                src = xT[c * 512:(c + 1) * 512, :].rearrange(
                    "(k p) t -> p k t", p=128)
                dst = xt_all[:, c * 4 * T:(c + 1) * 4 * T].rearrange(
                    "p (k t) -> p k t", k=4)
                _qs[c].dma_start(out=dst, in_=src)

        wv_all = ph1.tile([128, NK * KVCOLS], F32R, name="wv_all", tag="wv_all")
        wvt = [wv_all[:, k * KVCOLS:(k + 1) * KVCOLS] for k in range(NK)]
        nc.gpsimd.dma_start(
            out=wv_all[:, :].rearrange("p (k c) -> p k c", k=NK),
            in_=wv[:, :].rearrange("(k p) c -> p k c", p=128),
        )
        rope_c = ph1.tile([128, T], F32, name="rope_c", tag="rope_c")
        rope_s = ph1.tile([128, T], F32, name="rope_s", tag="rope_s")
        nc.gpsimd.dma_start(out=rope_c[:, :], in_=rope_c_d[:, :])
        nc.gpsimd.dma_start(out=rope_s[:, :], in_=rope_s_d[:, :])
        nc.gpsimd.dma_start(out=ones2[:, :], in_=ones2_d[:, :])
        mask_all = cp.tile([128, 4 * 512], F32, name="mask_all", tag="mask_all")
        nc.gpsimd.dma_start(
            out=mask_all[:, :].rearrange("p (q t) -> p q t", q=4),
            in_=masks_d[:, :, :].rearrange("q p t -> p q t"),
        )

        def rope(dst):
            # dst: [128, T] tile holding one 128-dim rope block in [d, t] layout
            tmp = rotp.tile([128, T], F32, name="rope_tmp", tag="rope_tmp", bufs=2)
            nc.sync.dma_start(out=tmp[0:64, :], in_=dst[64:128, :])
            nc.sync.dma_start(out=tmp[64:128, :], in_=dst[0:64, :])
            nc.vector.tensor_mul(tmp[:, :], tmp[:, :], rope_s[:, :])
            nc.vector.tensor_mul(r(dst[:, :]), dst[:, :], rope_c[:, :])
            nc.vector.tensor_add(r(dst[:, :]), dst[:, :], tmp[:, :])

        # Q^T / K^T: weight m-pairs loaded as [128, 256] tiles, DMAs spread
        # over the sync and scalar HWDGE queues
        def load_pair(w_dram, mp, eng):
            wt_all = ph1w.tile([128, NK * 256], F32R, name="wp_all", tag="wp_all", bufs=2)
            eng.dma_start(
                out=wt_all[:, :].rearrange("p (k c) -> p k c", k=NK),
                in_=w_dram[:, mp * 256:(mp + 1) * 256].rearrange("(k p) c -> p k c", p=128),
            )
            return [wt_all[:, k * 256:(k + 1) * 256] for k in range(NK)]

        def proj_pair(w_dram, dsts, mp, wt):
            for mi in range(2):
                dst = dsts[mi]
                msl = slice(mi * 128, (mi + 1) * 128)
                for t in range(NT):
                    ps = ph1ps.tile([128, 512], F32, name="proj_ps", tag="proj_ps", bufs=4)
                    for k in range(NK):
                        nc.tensor.matmul(
                            ps[:, :], wt[k][:, msl], xt[k][:, t * 512:(t + 1) * 512],
                            start=(k == 0), stop=(k == NK - 1),
                        )
                    nc.scalar.copy(r(dst[:, t * 512:(t + 1) * 512]), ps[:, :])
                rope(dst)

        pair_specs = [(wq, qT[2 * mp:2 * mp + 2], mp) for mp in range(MQ // 2)]
        pair_specs.append((wk, kT, 0))
        wt_next = load_pair(pair_specs[0][0], pair_specs[0][2], nc.scalar)
        load_xt()
        for pi, (w_dram, dsts, mp) in enumerate(pair_specs):
            wt = wt_next
            if pi + 1 < len(pair_specs):
                wt_next = load_pair(pair_specs[pi + 1][0], pair_specs[pi + 1][2],
                                    nc.sync if pi % 2 == 0 else nc.scalar)
            proj_pair(w_dram, dsts, mp, wt)

        # V in natural [t, dv] layout: lhsT = x^T chunk, rhs = wv chunk
        for i in range(NS):
            ps = ph1ps.tile([128, KVCOLS], F32, name="v_ps", tag="v_ps", bufs=2)
            for k in range(NK):
                nc.tensor.matmul(
                    ps[:, :],
                    xt[k][:, i * 128:(i + 1) * 128],
                    wvt[k][:, :],
                    start=(k == 0), stop=(k == NK - 1),
                )
            nc.scalar.copy(r(vt[i][:, :]), ps[:, :])

    # ---------------- phase 2: dual-branch attention ----------------
    op_cm = tc.tile_pool(name="oT", bufs=1)
    op = op_cm.__enter__()
    oT = [op.tile([128, T], F32, name=f"oT{i}", tag=f"oT{i}") for i in range(MQ)]

    with tc.tile_pool(name="pP", bufs=1) as pP, \
         tc.tile_pool(name="pR", bufs=2) as pR, \
         tc.tile_pool(name="scps", bufs=3, space="PSUM") as scps, \
         tc.tile_pool(name="dps", bufs=2, space="PSUM") as dpsp, \
         tc.tile_pool(name="avps", bufs=2, space="PSUM") as avpsp:

        def emit_branch(h, t, j, pt):
            # scores + exp + mask + denominator for one branch
            s_lim = 4 * (t + 1)
            tsl = slice(t * 512, (t + 1) * 512)
            qt = qT[2 * h + j]
            for si in range(s_lim):
                ps = scps.tile([128, 512], F32, name="sc_ps", tag="sc_ps", bufs=3)
                nc.tensor.matmul(
                    ps[:, :],
                    r(kT[j][:, si * 128:(si + 1) * 128]),
                    r(qt[:, tsl]),
                    start=True, stop=True,
                )
                nc.scalar.activation(
                    r(pt[j][si][:, :]), ps[:, :],
                    mybir.ActivationFunctionType.Exp, scale=SCALE,
                )
                rr = si - 4 * t
                if 0 <= rr < 4:
                    nc.vector.tensor_mul(r(pt[j][si][:, :]), pt[j][si][:, :], mask_all[:, rr * 512:(rr + 1) * 512])
            dps = dpsp.tile([1, 512], F32, name="d_ps", tag="d_ps", bufs=2)
            for si in range(s_lim):
                nc.tensor.matmul(
                    dps[:, :], ones2[:, 0:1], r(pt[j][si][:, :]),
                    start=(si == 0), stop=(si == s_lim - 1),
                )
            return dps

        def emit_ratio(h, t, d1ps, d2ps):
            # rb2 = lambda * d1/d2 broadcast, rb1 = 1/d1 broadcast
            rec1 = pR.tile([1, 512], F32, name="rec1", tag="rec1", bufs=2)
            nc.vector.reciprocal_approx_fast(rec1[:, :], d1ps[:, :])
            rec2 = pR.tile([1, 512], F32, name="rec2", tag="rec2", bufs=2)
            nc.vector.reciprocal_approx_fast(rec2[:, :], d2ps[:, :])
            c2 = pR.tile([1, 512], F32, name="c2", tag="c2", bufs=2)
            nc.vector.tensor_mul(c2[:, :], rec2[:, :], d1ps[:, :])
            nc.vector.tensor_scalar_mul(c2[:, :], c2[:, :], LAMBDA_INIT)
            rb1 = pR.tile([128, 512], F32, name="rb1", tag="rb1", bufs=2)
            nc.gpsimd.partition_broadcast(rb1[:, :], rec1[:, :])
            rb2 = pR.tile([128, 512], F32, name="rb2", tag="rb2", bufs=2)
            nc.gpsimd.partition_broadcast(rb2[:, :], c2[:, :])
            return rb1, rb2

        def emit_norm_av(h, t, pt, rb):
            # Pd = P1 - (lambda d1/d2) P2  (2 DVE ops per block), AV matmul,
            # then oT scaled by 1/d1 at copy-out
            rb1, rb2 = rb
            s_lim = 4 * (t + 1)
            tsl = slice(t * 512, (t + 1) * 512)
            for si in range(s_lim):
                nc.vector.tensor_mul(r(pt[1][si][:, :]), pt[1][si][:, :], rb2[:, :])
                nc.vector.tensor_sub(r(pt[0][si][:, :]), pt[0][si][:, :], pt[1][si][:, :])
            for dv in range(2):
                avps = avpsp.tile([128, 512], F32, name="av_ps", tag="av_ps", bufs=2)
                for si in range(s_lim):
                    nc.tensor.matmul(
                        avps[:, :],
                        r(vt[si][:, dv * 128:(dv + 1) * 128]),
                        r(pt[0][si][:, :]),
                        start=(si == 0), stop=(si == s_lim - 1),
                    )
                nc.vector.tensor_mul(r(oT[2 * h + dv][:, tsl]), avps[:, :], rb1[:, :])

        # software pipeline: AV of round r-1 slots after branch 0 of round r
        rounds = [(h, t) for h in range(HPC) for t in range(NT)]
        prev = None
        for (h, t) in rounds:
            s_lim = 4 * (t + 1)
            pt = [[pP.tile([128, 512], F32, name=f"p{j}_{si}", tag=f"p{j}_{si}", bufs=2)
                   for si in range(s_lim)] for j in range(2)]
            d1 = emit_branch(h, t, 0, pt)
            d2 = emit_branch(h, t, 1, pt)
            rb = emit_ratio(h, t, d1, d2)
            if prev is not None:
                emit_norm_av(*prev)
            prev = (h, t, pt, rb)
        emit_norm_av(*prev)

    # ---------------- phase 3: wo projection ----------------
    with tc.tile_pool(name="drampool", bufs=1, space="DRAM") as dram, \
         tc.tile_pool(name="wopool", bufs=2) as wop, \
         tc.tile_pool(name="wobounce", bufs=4) as wob, \
         tc.tile_pool(name="wops", bufs=4, space="PSUM") as wopsp:
        cdt = BF16 if with_collective else F32
        partial = [dram.tile([T, 512], cdt, name=f"partial{n}", tag=f"partial{n}")
                   for n in range(4)]
        rs_out = [dram.tile([T // 4, 512], cdt, name=f"rs_out{n}", tag=f"rs_out{n}")
                  for n in range(4)]
        for n in range(4):
            nsl = slice(n * 512, (n + 1) * 512)
            won = [wop.tile([128, 512], F32R, name=f"wo{fi}", tag=f"wo{fi}", bufs=2)
                   for fi in range(MQ)]
            for fi in range(MQ):
                nc.sync.dma_start(
                    out=won[fi][:, :],
                    in_=wo[fi * 128:(fi + 1) * 128, nsl],
                )
            for tm in range(NS):
                ps = wopsp.tile([128, 512], F32, name="wo_ps", tag="wo_ps", bufs=4)
                for fi in range(MQ):
                    nc.tensor.matmul(
                        ps[:, :],
                        r(oT[fi][:, tm * 128:(tm + 1) * 128]),
                        won[fi][:, :],
                        start=(fi == 0), stop=(fi == MQ - 1),
                    )
                bt = wob.tile([128, 512], cdt, name="wo_b", tag="wo_b", bufs=4)
                nc.scalar.copy(bt[:, :], ps[:, :])
                if with_collective:
                    nc.sync.dma_start(
                        out=partial[n][tm * 128:(tm + 1) * 128, :],
                        in_=bt[:, :],
                    )
                else:
                    nc.sync.dma_start(
                        out=out_d[tm * 128:(tm + 1) * 128, nsl],
                        in_=bt[:, :],
                    )
            if with_collective:
                # overlap: reduce-scatter this dout column block while the
                # next block's matmuls run; alternate queues so the two CC
                # rings work concurrently
                nc.gpsimd.collective_compute(
                    "ReduceScatter",
                    mybir.AluOpType.add,
                    replica_groups=GROUPS,
                    ins=[partial[n][:, :].opt()],
                    outs=[rs_out[n][:, :].opt()],
                )
                nc.gpsimd.dma_start(out=out_d[:, nsl], in_=rs_out[n][:, :])

    op_cm.__exit__(None, None, None)
    qp_cm.__exit__(None, None, None)
    cp_cm.__exit__(None, None, None)


def build(with_collective=True):
    nc = bacc.Bacc(
        "TRN2",
        target_bir_lowering=False,
        debug=False,
        num_devices=NCORES if with_collective else 1,
    )
    ins = [
        nc.dram_tensor("xT", [D, T], F32R, kind="ExternalInput").ap(),
        nc.dram_tensor("wq", [D, QCOLS], F32R, kind="ExternalInput").ap(),
        nc.dram_tensor("wk", [D, KVCOLS], F32R, kind="ExternalInput").ap(),
        nc.dram_tensor("wv", [D, KVCOLS], F32R, kind="ExternalInput").ap(),
        nc.dram_tensor("wo", [QCOLS, D], F32R, kind="ExternalInput").ap(),
        nc.dram_tensor("rope_c", [128, T], F32, kind="ExternalInput").ap(),
        nc.dram_tensor("rope_s", [128, T], F32, kind="ExternalInput").ap(),
        nc.dram_tensor("masks", [4, 128, 512], F32, kind="ExternalInput").ap(),
        nc.dram_tensor("ones2", [128, 2], F32R, kind="ExternalInput").ap(),
    ]
    oshape = [T // 4, D] if with_collective else [T, D]
    outs = [nc.dram_tensor("out", oshape, F32, kind="ExternalOutput").ap()]
    with tile.TileContext(nc) as tc:
        _body(tc, ins, outs, with_collective)
    nc.compile()
    return nc


def in_maps(x, wq, wk, wv, wo):
    rope_c, rope_s, masks, ones2 = _host_tables()
    x = np.asarray(x, np.float32)
    wq = np.asarray(wq, np.float32)
    wk = np.asarray(wk, np.float32)
    wv = np.asarray(wv, np.float32)
    wo = np.asarray(wo, np.float32)
    maps = []
    for c in range(NCORES):
        b, g = c // 4, c % 4
        maps.append({
            "xT": np.ascontiguousarray(x[b].T),
            "wq": np.ascontiguousarray(wq[:, g * QCOLS:(g + 1) * QCOLS]),
            "wk": np.ascontiguousarray(wk[:, g * KVCOLS:(g + 1) * KVCOLS]),
            "wv": np.ascontiguousarray(wv[:, g * KVCOLS:(g + 1) * KVCOLS]),
            "wo": np.ascontiguousarray(wo[g * QCOLS:(g + 1) * QCOLS, :]),
            "rope_c": rope_c,
            "rope_s": rope_s,
            "masks": masks,
            "ones2": ones2,
        })
    return maps


_NC_CACHE = {}


def get_nc(with_collective=True):
    key = bool(with_collective)
    if key not in _NC_CACHE:
        _NC_CACHE[key] = build(with_collective)
    return _NC_CACHE[key]


def kernel(x, wq, wk, wv, wo):
    from concourse.bass_utils import run_bass_kernel_spmd

    nc = get_nc(True)
    maps = in_maps(x, wq, wk, wv, wo)
    res = run_bass_kernel_spmd(nc, maps, core_ids=list(range(NCORES)))
    out = np.empty((B, T, D), np.float32)
    for c in range(NCORES):
        b, g = c // 4, c % 4
        out[b, g * (T // 4):(g + 1) * (T // 4), :] = res.results[c]["out"]
    return out


# revision 22
# speedup vs baseline: 1.2605x; 1.0018x over previous
"""DiffAttention (dual-branch differential attention) on 8 TRN2 NeuronCores.

Sharding: core c in 0..7 handles batch b = c//4 and kv-head group g = c%4
(4 query heads + 1 kv head per group, REP=4).  Each core computes its heads'
dual-branch attention locally in a transposed layout ([feature, t] on chip so
the head-dim contraction lands on the partition axis), projects through its
slice of wo, then a ReduceScatter over each batch's 4-core group sums the
partial outputs and leaves each core with a 256-row strip of the final
(1024, 2048) output.  The host stitches the strips together.

All matmuls run as float32r (full fp32 storage, fast PE mode).
"""

import sys

sys.path.insert(0, "/opt/trn_rl_repo")

import numpy as np

import concourse.bass as bass
import concourse.mybir as mybir
import concourse.tile as tile
from concourse import bacc

B, T, D = 2, 1024, 2048
H, KV, Dh = 16, 4, 128
REP = H // KV
LAMBDA_INIT = 0.2
ROPE_THETA = 10000.0
SCALE = 1.0 / float(np.sqrt(Dh))

NCORES = 8
GROUPS = [[0, 1, 2, 3], [4, 5, 6, 7]]

F32 = mybir.dt.float32
F32R = mybir.dt.float32r
BF16 = mybir.dt.bfloat16

HPC = 4          # query heads per core
MQ = 2 * HPC     # 8 q-feature chunks of 128 (q1/q2 per head)
NK = D // 128    # 16 contraction chunks for projections
NT = T // 512    # 2 t-chunks of 512
NS = T // 128    # 8 s-chunks of 128
QCOLS = 2 * HPC * Dh   # 1024 per-core q/wo feature columns
KVCOLS = 2 * Dh        # 256 per-core k/v columns


def r(ap):
    return ap.bitcast(F32R)


def _host_tables():
    """Transposed rope tables, causal diag masks, ones/fives columns."""
    inv_freq = 1.0 / (ROPE_THETA ** (np.arange(0, Dh, 2, dtype=np.float64) / Dh))
    freqs = np.arange(T, dtype=np.float64)[:, None] * inv_freq[None, :]  # (T, 64)
    cos = np.cos(freqs).astype(np.float32)
    sin = np.sin(freqs).astype(np.float32)
    rope_c = np.empty((Dh, T), np.float32)
    rope_s = np.empty((Dh, T), np.float32)
    rope_c[:64] = cos.T
    rope_c[64:] = cos.T
    rope_s[:64] = -sin.T
    rope_s[64:] = sin.T

    # mask[r][sp, tp] = 1 where (r*128 + sp) <= tp, for diagonal-crossing blocks
    masks = np.zeros((4, 128, 512), np.float32)
    sp = np.arange(128)[:, None]
    tp = np.arange(512)[None, :]
    for rr in range(4):
        masks[rr] = (rr * 128 + sp <= tp).astype(np.float32)

    ones2 = np.empty((128, 2), np.float32)
    ones2[:, 0] = 1.0
    ones2[:, 1] = 1.0 / LAMBDA_INIT  # 5.0: folds the lambda into 1/denom2
    return rope_c, rope_s, masks, ones2


def _body(tc, ins, outs, with_collective):
    nc = tc.nc
    xT, wq, wk, wv, wo, rope_c_d, rope_s_d, masks_d, ones2_d = ins
    out_d = outs[0]

    cp_cm = tc.tile_pool(name="const", bufs=1)
    cp = cp_cm.__enter__()
    ones2 = cp.tile([128, 2], F32R, name="ones2", tag="ones2")

    qp_cm = tc.tile_pool(name="qT", bufs=1)
    qp = qp_cm.__enter__()
    qT = [qp.tile([128, T], F32, name=f"qT{m}", tag=f"qT{m}") for m in range(MQ)]
    kT = [qp.tile([128, T], F32, name=f"kT{m}", tag=f"kT{m}") for m in range(2)]
    vt = [qp.tile([128, KVCOLS], F32, name=f"v{i}", tag=f"v{i}") for i in range(NS)]

    # ---------------- phase 1: projections + rope ----------------
    with tc.tile_pool(name="ph1", bufs=1) as ph1, \
         tc.tile_pool(name="ph1w", bufs=2) as ph1w, \
         tc.tile_pool(name="ph1ps", bufs=4, space="PSUM") as ph1ps, \
         tc.tile_pool(name="rotps", bufs=2) as rotp:
        # x^T: one tile per 128-row chunk so the first matmuls only wait on
        # the chunks they read; DMAs round-robin the three HWDGE queues
        xt = [ph1.tile([128, T], F32R, name=f"xt{k}", tag=f"xt{k}") for k in range(NK)]
        _qs = [nc.sync, nc.scalar, nc.gpsimd]

        def load_xt():
            for k in range(NK):
                _qs[k % 3].dma_start(out=xt[k][:, :], in_=xT[k * 128:(k + 1) * 128, :])

        wv_all = ph1.tile([128, NK * KVCOLS], F32R, name="wv_all", tag="wv_all")
        wvt = [wv_all[:, k * KVCOLS:(k + 1) * KVCOLS] for k in range(NK)]
        nc.gpsimd.dma_start(
            out=wv_all[:, :].rearrange("p (k c) -> p k c", k=NK),
            in_=wv[:, :].rearrange("(k p) c -> p k c", p=128),
        )
        rope_c = ph1.tile([128, T], F32, name="rope_c", tag="rope_c")
        rope_s = ph1.tile([128, T], F32, name="rope_s", tag="rope_s")
        nc.gpsimd.dma_start(out=rope_c[:, :], in_=rope_c_d[:, :])
        nc.gpsimd.dma_start(out=rope_s[:, :], in_=rope_s_d[:, :])
        nc.gpsimd.dma_start(out=ones2[:, :], in_=ones2_d[:, :])
        mask_all = cp.tile([128, 4 * 512], F32, name="mask_all", tag="mask_all")
        nc.gpsimd.dma_start(
            out=mask_all[:, :].rearrange("p (q t) -> p q t", q=4),
            in_=masks_d[:, :, :].rearrange("q p t -> p q t"),
        )

        def rope(dst):
            # dst: [128, T] tile holding one 128-dim rope block in [d, t] layout
            tmp = rotp.tile([128, T], F32, name="rope_tmp", tag="rope_tmp", bufs=2)
            nc.sync.dma_start(out=tmp[0:64, :], in_=dst[64:128, :])
            nc.sync.dma_start(out=tmp[64:128, :], in_=dst[0:64, :])
            nc.vector.tensor_mul(tmp[:, :], tmp[:, :], rope_s[:, :])
            nc.vector.tensor_mul(r(dst[:, :]), dst[:, :], rope_c[:, :])
            nc.vector.tensor_add(r(dst[:, :]), dst[:, :], tmp[:, :])

        # Q^T / K^T: weight m-pairs loaded as [128, 256] tiles, DMAs spread
        # over the sync and scalar HWDGE queues
        def load_pair(w_dram, mp, eng):
            wt_all = ph1w.tile([128, NK * 256], F32R, name="wp_all", tag="wp_all", bufs=2)
            eng.dma_start(
                out=wt_all[:, :].rearrange("p (k c) -> p k c", k=NK),
                in_=w_dram[:, mp * 256:(mp + 1) * 256].rearrange("(k p) c -> p k c", p=128),
            )
            return [wt_all[:, k * 256:(k + 1) * 256] for k in range(NK)]

        def proj_pair(w_dram, dsts, mp, wt):
            for mi in range(2):
                dst = dsts[mi]
                msl = slice(mi * 128, (mi + 1) * 128)
                for t in range(NT):
                    ps = ph1ps.tile([128, 512], F32, name="proj_ps", tag="proj_ps", bufs=4)
                    for k in range(NK):
                        nc.tensor.matmul(
                            ps[:, :], wt[k][:, msl], xt[k][:, t * 512:(t + 1) * 512],
                            start=(k == 0), stop=(k == NK - 1),
                        )
                    nc.scalar.copy(r(dst[:, t * 512:(t + 1) * 512]), ps[:, :])
                rope(dst)

        pair_specs = [(wq, qT[2 * mp:2 * mp + 2], mp) for mp in range(MQ // 2)]
        pair_specs.append((wk, kT, 0))
        wt_next = load_pair(pair_specs[0][0], pair_specs[0][2], nc.scalar)
        load_xt()
        for pi, (w_dram, dsts, mp) in enumerate(pair_specs):
            wt = wt_next
            if pi + 1 < len(pair_specs):
                wt_next = load_pair(pair_specs[pi + 1][0], pair_specs[pi + 1][2],
                                    nc.sync if pi % 2 == 0 else nc.scalar)
            proj_pair(w_dram, dsts, mp, wt)

        # V in natural [t, dv] layout: lhsT = x^T chunk, rhs = wv chunk
        for i in range(NS):
            ps = ph1ps.tile([128, KVCOLS], F32, name="v_ps", tag="v_ps", bufs=2)
            for k in range(NK):
                nc.tensor.matmul(
                    ps[:, :],
                    xt[k][:, i * 128:(i + 1) * 128],
                    wvt[k][:, :],
                    start=(k == 0), stop=(k == NK - 1),
                )
            nc.scalar.copy(r(vt[i][:, :]), ps[:, :])

    # ---------------- phase 2: dual-branch attention ----------------
    op_cm = tc.tile_pool(name="oT", bufs=1)
    op = op_cm.__enter__()
    oT = [op.tile([128, T], F32, name=f"oT{i}", tag=f"oT{i}") for i in range(MQ)]

    with tc.tile_pool(name="pP", bufs=1) as pP, \
         tc.tile_pool(name="pR", bufs=2) as pR, \
         tc.tile_pool(name="scps", bufs=3, space="PSUM") as scps, \
         tc.tile_pool(name="dps", bufs=2, space="PSUM") as dpsp, \
         tc.tile_pool(name="avps", bufs=2, space="PSUM") as avpsp:

        def emit_branch(h, t, j, pt):
            # scores + exp + mask + denominator for one branch
            s_lim = 4 * (t + 1)
            tsl = slice(t * 512, (t + 1) * 512)
            qt = qT[2 * h + j]
            for si in range(s_lim):
                ps = scps.tile([128, 512], F32, name="sc_ps", tag="sc_ps", bufs=3)
                nc.tensor.matmul(
                    ps[:, :],
                    r(kT[j][:, si * 128:(si + 1) * 128]),
                    r(qt[:, tsl]),
                    start=True, stop=True,
                )
                nc.scalar.activation(
                    r(pt[j][si][:, :]), ps[:, :],
                    mybir.ActivationFunctionType.Exp, scale=SCALE,
                )
                rr = si - 4 * t
                if 0 <= rr < 4:
                    nc.vector.tensor_mul(r(pt[j][si][:, :]), pt[j][si][:, :], mask_all[:, rr * 512:(rr + 1) * 512])
            dps = dpsp.tile([1, 512], F32, name="d_ps", tag="d_ps", bufs=2)
            for si in range(s_lim):
                nc.tensor.matmul(
                    dps[:, :], ones2[:, 0:1], r(pt[j][si][:, :]),
                    start=(si == 0), stop=(si == s_lim - 1),
                )
            return dps

        def emit_ratio(h, t, d1ps, d2ps):
            # rb2 = lambda * d1/d2 broadcast, rb1 = 1/d1 broadcast
            rec1 = pR.tile([1, 512], F32, name="rec1", tag="rec1", bufs=2)
            nc.vector.reciprocal_approx_fast(rec1[:, :], d1ps[:, :])
            rec2 = pR.tile([1, 512], F32, name="rec2", tag="rec2", bufs=2)
            nc.vector.reciprocal_approx_fast(rec2[:, :], d2ps[:, :])
            c2 = pR.tile([1, 512], F32, name="c2", tag="c2", bufs=2)
            nc.vector.tensor_mul(c2[:, :], rec2[:, :], d1ps[:, :])
            nc.vector.tensor_scalar_mul(c2[:, :], c2[:, :], LAMBDA_INIT)
            rb1 = pR.tile([128, 512], F32, name="rb1", tag="rb1", bufs=2)
            nc.gpsimd.partition_broadcast(rb1[:, :], rec1[:, :])
            rb2 = pR.tile([128, 512], F32, name="rb2", tag="rb2", bufs=2)
            nc.gpsimd.partition_broadcast(rb2[:, :], c2[:, :])
            return rb1, rb2

        def emit_norm_av(h, t, pt, rb):
            # Pd = P1 - (lambda d1/d2) P2  (2 DVE ops per block), AV matmul,
            # then oT scaled by 1/d1 at copy-out
            rb1, rb2 = rb
            s_lim = 4 * (t + 1)
            tsl = slice(t * 512, (t + 1) * 512)
            for si in range(s_lim):
                nc.vector.tensor_mul(r(pt[1][si][:, :]), pt[1][si][:, :], rb2[:, :])
                nc.vector.tensor_sub(r(pt[0][si][:, :]), pt[0][si][:, :], pt[1][si][:, :])
            for dv in range(2):
                avps = avpsp.tile([128, 512], F32, name="av_ps", tag="av_ps", bufs=2)
                for si in range(s_lim):
                    nc.tensor.matmul(
                        avps[:, :],
                        r(vt[si][:, dv * 128:(dv + 1) * 128]),
                        r(pt[0][si][:, :]),
                        start=(si == 0), stop=(si == s_lim - 1),
                    )
                nc.vector.tensor_mul(r(oT[2 * h + dv][:, tsl]), avps[:, :], rb1[:, :])

        # software pipeline: AV of round r-1 slots after branch 0 of round r
        rounds = [(h, t) for h in range(HPC) for t in range(NT)]
        prev = None
        for (h, t) in rounds:
            s_lim = 4 * (t + 1)
            pt = [[pP.tile([128, 512], F32, name=f"p{j}_{si}", tag=f"p{j}_{si}", bufs=2)
                   for si in range(s_lim)] for j in range(2)]
            d1 = emit_branch(h, t, 0, pt)
            d2 = emit_branch(h, t, 1, pt)
            rb = emit_ratio(h, t, d1, d2)
            if prev is not None:
                emit_norm_av(*prev)
            prev = (h, t, pt, rb)
        emit_norm_av(*prev)

    # ---------------- phase 3: wo projection ----------------
    with tc.tile_pool(name="drampool", bufs=1, space="DRAM") as dram, \
         tc.tile_pool(name="wopool", bufs=2) as wop, \
         tc.tile_pool(name="wobounce", bufs=4) as wob, \
         tc.tile_pool(name="wops", bufs=4, space="PSUM") as wopsp:
        cdt = BF16 if with_collective else F32
        partial = [dram.tile([T, 1024], cdt, name=f"partial{n}", tag=f"partial{n}")
                   for n in range(2)]
        rs_out = [dram.tile([T // 4, 1024], cdt, name=f"rs_out{n}", tag=f"rs_out{n}")
                  for n in range(2)]
        for n in range(4):
            nsl = slice(n * 512, (n + 1) * 512)
            won = [wop.tile([128, 512], F32R, name=f"wo{fi}", tag=f"wo{fi}", bufs=2)
                   for fi in range(MQ)]
            for fi in range(MQ):
                nc.sync.dma_start(
                    out=won[fi][:, :],
                    in_=wo[fi * 128:(fi + 1) * 128, nsl],
                )
            for tm in range(NS):
                ps = wopsp.tile([128, 512], F32, name="wo_ps", tag="wo_ps", bufs=4)
                for fi in range(MQ):
                    nc.tensor.matmul(
                        ps[:, :],
                        r(oT[fi][:, tm * 128:(tm + 1) * 128]),
                        won[fi][:, :],
                        start=(fi == 0), stop=(fi == MQ - 1),
                    )
                bt = wob.tile([128, 512], cdt, name="wo_b", tag="wo_b", bufs=4)
                nc.scalar.copy(bt[:, :], ps[:, :])
                if with_collective:
                    nc.sync.dma_start(
                        out=partial[n // 2][tm * 128:(tm + 1) * 128,
                                            (n % 2) * 512:(n % 2) * 512 + 512],
                        in_=bt[:, :],
                    )
                else:
                    nc.sync.dma_start(
                        out=out_d[tm * 128:(tm + 1) * 128, nsl],
                        in_=bt[:, :],
                    )
            if with_collective and n % 2 == 1:
                # overlap: reduce-scatter each 1024-col half while the other
                # half's matmuls run
                c = n // 2
                csl = slice(c * 1024, (c + 1) * 1024)
                nc.gpsimd.collective_compute(
                    "ReduceScatter",
                    mybir.AluOpType.add,
                    replica_groups=GROUPS,
                    ins=[partial[c][:, :].opt()],
                    outs=[rs_out[c][:, :].opt()],
                )
                nc.gpsimd.dma_start(out=out_d[:, csl], in_=rs_out[c][:, :])

    op_cm.__exit__(None, None, None)
    qp_cm.__exit__(None, None, None)
    cp_cm.__exit__(None, None, None)


def build(with_collective=True):
    nc = bacc.Bacc(
        "TRN2",
        target_bir_lowering=False,
        debug=False,
        num_devices=NCORES if with_collective else 1,
    )
    ins = [
        nc.dram_tensor("xT", [D, T], F32R, kind="ExternalInput").ap(),
        nc.dram_tensor("wq", [D, QCOLS], F32R, kind="ExternalInput").ap(),
        nc.dram_tensor("wk", [D, KVCOLS], F32R, kind="ExternalInput").ap(),
        nc.dram_tensor("wv", [D, KVCOLS], F32R, kind="ExternalInput").ap(),
        nc.dram_tensor("wo", [QCOLS, D], F32R, kind="ExternalInput").ap(),
        nc.dram_tensor("rope_c", [128, T], F32, kind="ExternalInput").ap(),
        nc.dram_tensor("rope_s", [128, T], F32, kind="ExternalInput").ap(),
        nc.dram_tensor("masks", [4, 128, 512], F32, kind="ExternalInput").ap(),
        nc.dram_tensor("ones2", [128, 2], F32R, kind="ExternalInput").ap(),
    ]
    oshape = [T // 4, D] if with_collective else [T, D]
    outs = [nc.dram_tensor("out", oshape, F32, kind="ExternalOutput").ap()]
    with tile.TileContext(nc) as tc:
        _body(tc, ins, outs, with_collective)
    nc.compile()
    return nc


def in_maps(x, wq, wk, wv, wo):
    rope_c, rope_s, masks, ones2 = _host_tables()
    x = np.asarray(x, np.float32)
    wq = np.asarray(wq, np.float32)
    wk = np.asarray(wk, np.float32)
    wv = np.asarray(wv, np.float32)
    wo = np.asarray(wo, np.float32)
    maps = []
    for c in range(NCORES):
        b, g = c // 4, c % 4
        maps.append({
            "xT": np.ascontiguousarray(x[b].T),
            "wq": np.ascontiguousarray(wq[:, g * QCOLS:(g + 1) * QCOLS]),
            "wk": np.ascontiguousarray(wk[:, g * KVCOLS:(g + 1) * KVCOLS]),
            "wv": np.ascontiguousarray(wv[:, g * KVCOLS:(g + 1) * KVCOLS]),
            "wo": np.ascontiguousarray(wo[g * QCOLS:(g + 1) * QCOLS, :]),
            "rope_c": rope_c,
            "rope_s": rope_s,
            "masks": masks,
            "ones2": ones2,
        })
    return maps


_NC_CACHE = {}


def get_nc(with_collective=True):
    key = bool(with_collective)
    if key not in _NC_CACHE:
        _NC_CACHE[key] = build(with_collective)
    return _NC_CACHE[key]


def kernel(x, wq, wk, wv, wo):
    from concourse.bass_utils import run_bass_kernel_spmd

    nc = get_nc(True)
    maps = in_maps(x, wq, wk, wv, wo)
    res = run_bass_kernel_spmd(nc, maps, core_ids=list(range(NCORES)))
    out = np.empty((B, T, D), np.float32)
    for c in range(NCORES):
        b, g = c // 4, c % 4
        out[b, g * (T // 4):(g + 1) * (T // 4), :] = res.results[c]["out"]
    return out
